# revision 18
# baseline (speedup 1.0000x reference)
"""AttnPooling Trainium2 kernel: 8-core data-parallel over B*N items.

Per item (b,n): x is (D=128, K=4096) fp32, K contiguous in DRAM.
  mean   = sum_k x[:,k]*m[k] / c           (c = sum m)
  query  = Wq @ mean + bq
  v      = Wk^T query = (Wk^T Wq) mean + Wk^T bq
  s_k    = v . x_k   (+ const that cancels in softmax; bk unused)
  p_k    = m_k exp(s_k/sqrt(D));  out = sum_k p_k x_k / sum_k p_k

Structure (v2): the mask broadcast mb and the exp broadcast eb are built by
ones-matmuls on the tensor engine (PE) into PSUM; the vector engine runs two
TTR passes per item (TTR1: xm = x*mb bf16 + masked-sum accum -> c*mean;
TTR2: xm*eb accum -> pooled numerator). Scores s'_k = vt.xm_k are 0 at
masked k so exp gives 1 there; Z is corrected by Zfake - (K - c). Scalar
engine does only the exp (4 chunks of 1024 per item). x DMA is prefetched
2 items deep; items are software-pipelined lag-1 (mean(i) || attn(i-1)).
"""

import sys

sys.path.insert(0, "/opt/trn_rl_repo")

import numpy as np
from contextlib import ExitStack

NI = 16  # items per core
D = 128
K = 4096
NCORES = 8
SD = 1.0 / np.sqrt(128.0)  # 1/sqrt(D)
CH = 1024  # TTR / exp chunk
NCH = K // CH

_CACHE = {}


def _build():
    import concourse.bass as bass
    import concourse.tile as tile
    from concourse import bacc, mybir
    from concourse.dve_ops import TENSOR_TENSOR_REDUCE

    dt = mybir.dt
    Alu = mybir.AluOpType
    Act = mybir.ActivationFunctionType

    nc = bacc.Bacc(
        "TRN2", target_bir_lowering=False, debug=False, num_devices=NCORES
    )
    x_d = nc.dram_tensor("x", [NI, D, K], dt.float32, kind="ExternalInput").ap()
    mrow_d = nc.dram_tensor("mrow", [NI, K], dt.bfloat16, kind="ExternalInput").ap()
    crow_d = nc.dram_tensor("crow", [1, NI], dt.float32, kind="ExternalInput").ap()
    cinv_d = nc.dram_tensor("cinv", [1, NI], dt.float32, kind="ExternalInput").ap()
    wq_d = nc.dram_tensor("Wq", [D, D], dt.float32, kind="ExternalInput").ap()
    wk_d = nc.dram_tensor("Wk", [D, D], dt.float32, kind="ExternalInput").ap()
    bq_d = nc.dram_tensor("bq", [D, 1], dt.float32, kind="ExternalInput").ap()
    out_d = nc.dram_tensor("out", [D, NI], dt.float32, kind="ExternalOutput").ap()

    with tile.TileContext(nc) as tc, ExitStack() as ctx:
        # SBUF pools
        xp = ctx.enter_context(tc.tile_pool(name="xp", bufs=3))
        xmp = ctx.enter_context(tc.tile_pool(name="xmp", bufs=3))
        junk = ctx.enter_context(tc.tile_pool(name="junk", bufs=2))
        mbsp = ctx.enter_context(tc.tile_pool(name="mbsp", bufs=3))
        per = ctx.enter_context(tc.tile_pool(name="per", bufs=1))
        # PSUM pool: tag w1 (mb) 2bufs*2banks + tag w2 (st/eb/small) 2bufs*2banks
        pp = ctx.enter_context(tc.tile_pool(name="pp", bufs=2, space="PSUM"))

        # persistent tiles
        wq = per.tile([D, D], dt.float32, tag="wq")
        wk = per.tile([D, D], dt.float32, tag="wk")
        bq = per.tile([D, 1], dt.float32, tag="bq")
        cqk = per.tile([D, D], dt.bfloat16, tag="cqk")
        meanb = per.tile([D, NI], dt.bfloat16, tag="meanb")
        w0 = per.tile([D, 1], dt.float32, tag="w0")
        ones32 = per.tile([32, D], dt.bfloat16, tag="ones32")
        ones32f = per.tile([32, D], dt.float32, tag="ones32f")
        # persistent (1, K) mask-row carriers at partition 0
        m0s = [
            per.tile([1, K], dt.bfloat16, tag=f"m0_{j}", name=f"m0_{j}")
            for j in range(3)
        ]
        # persistent (32, K) row-carrier tiles: row 0 live, rows 1-31 stay 0
        ets = [
            per.tile([32, K], dt.bfloat16, tag=f"et_{j}", name=f"et_{j}")
            for j in range(3)
        ]
        cinvrow32 = per.tile([32, NI], dt.float32, tag="cinvrow32")
        cinvb = per.tile([D, NI], dt.float32, tag="cinvb")
        crow_t = per.tile([1, NI], dt.float32, tag="crow_t")
        zp = per.tile([1, NI * NCH], dt.float32, tag="zp")
        mean = per.tile([D, NI], dt.float32, tag="mean")
        vt = per.tile([D, NI], dt.bfloat16, tag="vt")
        praw4 = per.tile([D, NCH * NI], dt.float32, tag="praw4")
        praw = per.tile([D, NI], dt.float32, tag="praw")
        zrow = per.tile([1, NI], dt.float32, tag="zrow")
        zinv = per.tile([1, NI], dt.float32, tag="zinv")
        frow32 = per.tile([32, NI], dt.float32, tag="frow32")
        outt = per.tile([D, NI], dt.float32, tag="outt")

        # ---- setup ----
        nc.sync.dma_start(wq[:, :], wq_d[:, :])
        nc.sync.dma_start(wk[:, :], wk_d[:, :])
        nc.sync.dma_start(bq[:, :], bq_d[:, :])
        nc.sync.dma_start(crow_t[:, :], crow_d[:, :])
        nc.vector.memset(ones32[:, :], 0.0)
        nc.vector.memset(ones32[0:1, :], 1.0)
        nc.vector.memset(ones32f[:, :], 0.0)
        nc.vector.memset(ones32f[0:1, :], 1.0)
        nc.vector.memset(cinvrow32[:, :], 0.0)
        nc.vector.memset(frow32[:, :], 0.0)
        nc.sync.dma_start(cinvrow32[0:1, :], cinv_d[:, :])
        crow = crow_t[0:1, 0:NI]

        def setup2():
            # weights preprocessing, emitted after TTR1(0) so it does not
            # gate the first mean pass
            cinvb_ps = pp.tile([D, NI], dt.float32, tag="w2")
            nc.tensor.matmul(
                cinvb_ps[:, :],
                ones32f[:, :],
                cinvrow32[:, :],
                start=True,
                stop=True,
            )
            nc.scalar.copy(cinvb[:, :], cinvb_ps[:, :])
            # CQK = Wq^T Wk ; w0 = Wk^T bq
            cqk_ps = pp.tile([D, D], dt.float32, tag="w2")
            nc.tensor.matmul(
                cqk_ps[:, :], wq[:, :], wk[:, :], start=True, stop=True
            )
            nc.scalar.copy(cqk[:, :], cqk_ps[:, :])
            w0_ps = pp.tile([D, 1], dt.float32, tag="w2")
            nc.tensor.matmul(
                w0_ps[:, :], wk[:, :], bq[:, :], start=True, stop=True
            )
            nc.scalar.copy(w0[:, :], w0_ps[:, :])

        # ---- per-item software pipeline (lag-1: mean(i) || attn(i-1)) ----
        xts = [None] * NI
        xms = [None] * NI
        mbSs = [None] * NI

        def prefetch(i):
            xt = xp.tile([D, K], dt.float32, tag="x", name=f"x_{i}")
            xts[i] = xt
            nc.sync.dma_start(xt[:, :], x_d[i, :, :])

        def mask_bcast(i):
            # mask row -> partition 0 carrier, then Pool broadcasts to SBUF
            m0 = m0s[i % 3]
            nc.sync.dma_start(m0[0:1, :], mrow_d[i : i + 1, :])
            mbS = mbsp.tile([D, K], dt.bfloat16, tag="mb", name=f"mbS_{i}")
            mbSs[i] = mbS
            nc.gpsimd.partition_broadcast(mbS[:, :], m0[0:1, :])

        def slot(i):
            # emission order = per-engine readiness: PE gets the ready scores
            # matmuls of item i-1 first, DVE gets TTR1(i) first, then the
            # exp/eb/TTR2 chain of i-1, and the v-chain of i at the tail.
            j = i - 1  # attn item
            if j >= 0:
                xmj = xms[j]
                et = ets[j % 3]
                sts = []
                for c in range(NCH):
                    st = pp.tile(
                        [1, CH], dt.float32, tag="w2", name=f"st_{j}_{c}"
                    )
                    sts.append(st)
                    for h in range(CH // 512):
                        lo = c * CH + h * 512
                        nc.tensor.matmul(
                            st[:, h * 512 : (h + 1) * 512],
                            vt[:, j : j + 1],
                            xmj[:, lo : lo + 512],
                            start=True,
                            stop=True,
                        )
            if i < NI:
                xt = xts[i]
                xm = xmp.tile([D, K], dt.bfloat16, tag="xm", name=f"xm_{i}")
                xms[i] = xm
                nc.vector._custom_dve(
                    TENSOR_TENSOR_REDUCE,
                    out=xm[:, :],
                    in0=xt[:, :],
                    in1=mbSs[i][:, :],
                    s0=0.0,
                    s1=1.0,
                    accum_out=mean[:, i : i + 1],
                )
                if i == 0:
                    setup2()
            if j >= 0:
                for c in range(NCH):
                    nc.scalar.activation(
                        et[0:1, c * CH : (c + 1) * CH],
                        sts[c][:, :],
                        Act.Exp,
                        scale=SD,
                        accum_out=zp[0:1, NCH * j + c : NCH * j + c + 1],
                    )
                    eb = pp.tile(
                        [D, CH], dt.float32, tag="w1", name=f"eb_{j}_{c}"
                    )
                    for h in range(CH // 512):
                        lo = c * CH + h * 512
                        nc.tensor.matmul(
                            eb[:, h * 512 : (h + 1) * 512],
                            ones32[:, :],
                            et[:, lo : lo + 512],
                            start=True,
                            stop=True,
                        )
                    jt = junk.tile(
                        [D, CH], dt.bfloat16, tag="junk", name=f"j_{j}_{c}"
                    )
                    nc.vector._custom_dve(
                        TENSOR_TENSOR_REDUCE,
                        out=jt[:, :],
                        in0=xmj[:, c * CH : (c + 1) * CH],
                        in1=eb[:, :],
                        s0=0.0,
                        s1=1.0,
                        accum_out=praw4[:, NCH * j + c : NCH * j + c + 1],
                    )
            if i < NI:
                # v_i = (CQK^T . u_i) * (1/c_i) + w0
                nc.scalar.copy(meanb[:, i : i + 1], mean[:, i : i + 1])
                vps = pp.tile([D, 1], dt.float32, tag="w2", name=f"vps_{i}")
                nc.tensor.matmul(
                    vps[:, :],
                    cqk[:, :],
                    meanb[:, i : i + 1],
                    start=True,
                    stop=True,
                )
                nc.scalar.activation(
                    vt[:, i : i + 1],
                    vps[:, :],
                    Act.Identity,
                    bias=w0[:, 0:1],
                    scale=cinvb[:, i : i + 1],
                )

        prefetch(0)
        prefetch(1)
        mask_bcast(0)
        for t in ets:
            nc.vector.memset(t[:, :], 0.0)
        mask_bcast(1)
        for i in range(NI + 1):
            if i + 2 < NI:
                prefetch(i + 2)
            if i + 2 < NI:
                mask_bcast(i + 2)
            slot(i)

        # ---- finalize: out = praw / Z with Z = Zfake - (K - c) ----
        p4 = praw4[:, :].rearrange("p (n c) -> p n c", c=NCH)
        nc.vector.tensor_reduce(
            praw[:, :], p4, axis=mybir.AxisListType.X, op=Alu.add
        )
        zp3 = zp[:, :].rearrange("a (n c) -> a n c", c=NCH)
        nc.vector.tensor_reduce(
            zrow[:, :], zp3, axis=mybir.AxisListType.X, op=Alu.add
        )
        # zrow = (zfake + c) - K
        nc.vector.tensor_tensor(zrow[:, :], zrow[:, :], crow, op=Alu.add)
        nc.vector.tensor_scalar(
            zrow[:, :], zrow[:, :], -float(K), None, op0=Alu.add
        )
        nc.vector.reciprocal(zinv[:, :], zrow[:, :])
        nc.vector.tensor_copy(frow32[0:1, :], zinv[:, :])
        fb = pp.tile([D, NI], dt.float32, tag="w2")
        nc.tensor.matmul(
            fb[:, :], ones32f[:, :], frow32[:, :], start=True, stop=True
        )
        nc.vector.tensor_tensor(outt[:, :], praw[:, :], fb[:, :], op=Alu.mult)
        nc.sync.dma_start(out_d[:, :], outt[:, :])

    nc.compile()
    return nc


def _get_nc():
    if "nc" not in _CACHE:
        _CACHE["nc"] = _build()
    return _CACHE["nc"]


def kernel(x, mask, Wq, bq, Wk, bk):
    import ml_dtypes
    from concourse.bass_utils import run_bass_kernel_spmd

    nc = _get_nc()
    B, N, d, H, W = x.shape
    xr = np.ascontiguousarray(x.reshape(B * N, d, H * W).astype(np.float32))
    mr = mask.reshape(B * N, H * W)
    mrow = np.ascontiguousarray(mr.astype(ml_dtypes.bfloat16))
    cnt = np.asarray(mr).astype(np.float32).sum(axis=1)
    bq2 = np.ascontiguousarray(bq.reshape(d, 1).astype(np.float32))
    wqc = np.ascontiguousarray(Wq.astype(np.float32))
    wkc = np.ascontiguousarray(Wk.astype(np.float32))
    in_maps = []
    for c in range(NCORES):
        s = slice(c * NI, (c + 1) * NI)
        in_maps.append(
            {
                "x": np.ascontiguousarray(xr[s]),
                "mrow": np.ascontiguousarray(mrow[s]),
                "crow": np.ascontiguousarray(cnt[s].reshape(1, NI)),
                "cinv": np.ascontiguousarray((1.0 / cnt[s]).reshape(1, NI)),
                "Wq": wqc,
                "Wk": wkc,
                "bq": bq2,
            }
        )
    res = run_bass_kernel_spmd(nc, in_maps, core_ids=list(range(NCORES)))
    parts = [np.asarray(res.results[c]["out"]).T for c in range(NCORES)]
    return np.concatenate(parts, axis=0).reshape(B, N, d).astype(np.float32)


# revision 19
# speedup vs baseline: 1.0090x; 1.0090x over previous
"""AttnPooling Trainium2 kernel: 8-core data-parallel over B*N items.

Per item (b,n): x is (D=128, K=4096) fp32, K contiguous in DRAM.
  mean   = sum_k x[:,k]*m[k] / c           (c = sum m)
  query  = Wq @ mean + bq
  v      = Wk^T query = (Wk^T Wq) mean + Wk^T bq
  s_k    = v . x_k   (+ const that cancels in softmax; bk unused)
  p_k    = m_k exp(s_k/sqrt(D));  out = sum_k p_k x_k / sum_k p_k

Structure (v2): the mask broadcast mb and the exp broadcast eb are built by
ones-matmuls on the tensor engine (PE) into PSUM; the vector engine runs two
TTR passes per item (TTR1: xm = x*mb bf16 + masked-sum accum -> c*mean;
TTR2: xm*eb accum -> pooled numerator). Scores s'_k = vt.xm_k are 0 at
masked k so exp gives 1 there; Z is corrected by Zfake - (K - c). Scalar
engine does only the exp (4 chunks of 1024 per item). x DMA is prefetched
2 items deep; items are software-pipelined lag-1 (mean(i) || attn(i-1)).
"""

import sys

sys.path.insert(0, "/opt/trn_rl_repo")

import numpy as np
from contextlib import ExitStack

NI = 16  # items per core
D = 128
K = 4096
NCORES = 8
SD = 1.0 / np.sqrt(128.0)  # 1/sqrt(D)
CH = 1024  # TTR / exp chunk
NCH = K // CH

_CACHE = {}


def _build():
    import concourse.bass as bass
    import concourse.tile as tile
    from concourse import bacc, mybir
    from concourse.dve_ops import TENSOR_TENSOR_REDUCE

    dt = mybir.dt
    Alu = mybir.AluOpType
    Act = mybir.ActivationFunctionType

    nc = bacc.Bacc(
        "TRN2", target_bir_lowering=False, debug=False, num_devices=NCORES
    )
    x_d = nc.dram_tensor("x", [NI, D, K], dt.float32, kind="ExternalInput").ap()
    mrow_d = nc.dram_tensor("mrow", [NI, K], dt.bfloat16, kind="ExternalInput").ap()
    crow_d = nc.dram_tensor("crow", [1, NI], dt.float32, kind="ExternalInput").ap()
    cinv_d = nc.dram_tensor("cinv", [1, NI], dt.float32, kind="ExternalInput").ap()
    wq_d = nc.dram_tensor("Wq", [D, D], dt.float32, kind="ExternalInput").ap()
    wk_d = nc.dram_tensor("Wk", [D, D], dt.float32, kind="ExternalInput").ap()
    bq_d = nc.dram_tensor("bq", [D, 1], dt.float32, kind="ExternalInput").ap()
    out_d = nc.dram_tensor("out", [D, NI], dt.float32, kind="ExternalOutput").ap()

    with tile.TileContext(nc) as tc, ExitStack() as ctx:
        # SBUF pools
        xp = ctx.enter_context(tc.tile_pool(name="xp", bufs=3))
        xmp = ctx.enter_context(tc.tile_pool(name="xmp", bufs=3))
        junk = ctx.enter_context(tc.tile_pool(name="junk", bufs=2))
        mbsp = ctx.enter_context(tc.tile_pool(name="mbsp", bufs=3))
        per = ctx.enter_context(tc.tile_pool(name="per", bufs=1))
        # PSUM pool: tag w1 (mb) 2bufs*2banks + tag w2 (st/eb/small) 2bufs*2banks
        pp = ctx.enter_context(tc.tile_pool(name="pp", bufs=2, space="PSUM"))

        # persistent tiles
        wq = per.tile([D, D], dt.float32, tag="wq")
        wk = per.tile([D, D], dt.float32, tag="wk")
        bq = per.tile([D, 1], dt.float32, tag="bq")
        cqk = per.tile([D, D], dt.bfloat16, tag="cqk")
        meanb = per.tile([D, NI], dt.bfloat16, tag="meanb")
        w0 = per.tile([D, 1], dt.float32, tag="w0")
        ones32 = per.tile([32, D], dt.bfloat16, tag="ones32")
        ones32f = per.tile([32, D], dt.float32, tag="ones32f")
        # persistent (1, K) mask-row carriers at partition 0
        m0s = [
            per.tile([1, K], dt.bfloat16, tag=f"m0_{j}", name=f"m0_{j}")
            for j in range(3)
        ]
        # persistent (32, K) row-carrier tiles: row 0 live, rows 1-31 stay 0
        ets = [
            per.tile([32, K], dt.bfloat16, tag=f"et_{j}", name=f"et_{j}")
            for j in range(3)
        ]
        cinvrow32 = per.tile([32, NI], dt.float32, tag="cinvrow32")
        cinvb = per.tile([D, NI], dt.float32, tag="cinvb")
        crow_t = per.tile([1, NI], dt.float32, tag="crow_t")
        zp = per.tile([1, NI * NCH], dt.float32, tag="zp")
        mean = per.tile([D, NI], dt.float32, tag="mean")
        vt = per.tile([D, NI], dt.bfloat16, tag="vt")
        praw4 = per.tile([D, NCH * NI], dt.float32, tag="praw4")
        praw = per.tile([D, NI], dt.float32, tag="praw")
        zrow = per.tile([1, NI], dt.float32, tag="zrow")
        zinv = per.tile([1, NI], dt.float32, tag="zinv")
        frow32 = per.tile([32, NI], dt.float32, tag="frow32")
        outt = per.tile([D, NI], dt.float32, tag="outt")

        # ---- setup ----
        nc.sync.dma_start(wq[:, :], wq_d[:, :])
        nc.sync.dma_start(wk[:, :], wk_d[:, :])
        nc.sync.dma_start(bq[:, :], bq_d[:, :])
        nc.sync.dma_start(crow_t[:, :], crow_d[:, :])
        nc.vector.memset(ones32[:, :], 0.0)
        nc.vector.memset(ones32[0:1, :], 1.0)
        nc.vector.memset(ones32f[:, :], 0.0)
        nc.vector.memset(ones32f[0:1, :], 1.0)
        nc.vector.memset(cinvrow32[:, :], 0.0)
        nc.vector.memset(frow32[:, :], 0.0)
        nc.sync.dma_start(cinvrow32[0:1, :], cinv_d[:, :])
        crow = crow_t[0:1, 0:NI]

        def setup2():
            # weights preprocessing, emitted after TTR1(0) so it does not
            # gate the first mean pass
            cinvb_ps = pp.tile([D, NI], dt.float32, tag="w2")
            nc.tensor.matmul(
                cinvb_ps[:, :],
                ones32f[:, :],
                cinvrow32[:, :],
                start=True,
                stop=True,
            )
            nc.scalar.copy(cinvb[:, :], cinvb_ps[:, :])
            # CQK = Wq^T Wk ; w0 = Wk^T bq
            cqk_ps = pp.tile([D, D], dt.float32, tag="w2")
            nc.tensor.matmul(
                cqk_ps[:, :], wq[:, :], wk[:, :], start=True, stop=True
            )
            nc.scalar.copy(cqk[:, :], cqk_ps[:, :])
            w0_ps = pp.tile([D, 1], dt.float32, tag="w2")
            nc.tensor.matmul(
                w0_ps[:, :], wk[:, :], bq[:, :], start=True, stop=True
            )
            nc.scalar.copy(w0[:, :], w0_ps[:, :])

        # ---- per-item software pipeline (lag-1: mean(i) || attn(i-1)) ----
        xts = [None] * NI
        xms = [None] * NI
        mbSs = [None] * NI

        def prefetch(i):
            xt = xp.tile([D, K], dt.float32, tag="x", name=f"x_{i}")
            xts[i] = xt
            nc.sync.dma_start(xt[:, :], x_d[i, :, :])

        def mask_bcast(i):
            # mask row -> partition 0 carrier, then Pool broadcasts to SBUF
            m0 = m0s[i % 3]
            nc.sync.dma_start(m0[0:1, :], mrow_d[i : i + 1, :])
            mbS = mbsp.tile([D, K], dt.bfloat16, tag="mb", name=f"mbS_{i}")
            mbSs[i] = mbS
            nc.gpsimd.partition_broadcast(mbS[:, :], m0[0:1, :])

        def slot(i):
            # emission order = per-engine readiness: PE gets the ready scores
            # matmuls of item i-1 first, DVE gets TTR1(i) first, then the
            # exp/eb/TTR2 chain of i-1, and the v-chain of i at the tail.
            j = i - 1  # attn item
            if j >= 0:
                xmj = xms[j]
                et = ets[j % 3]
                sts = []
                for c in range(NCH):
                    st = pp.tile(
                        [1, CH], dt.float32, tag="w2", name=f"st_{j}_{c}"
                    )
                    sts.append(st)
                    for h in range(CH // 512):
                        lo = c * CH + h * 512
                        nc.tensor.matmul(
                            st[:, h * 512 : (h + 1) * 512],
                            vt[:, j : j + 1],
                            xmj[:, lo : lo + 512],
                            start=True,
                            stop=True,
                        )
            if i < NI:
                xt = xts[i]
                xm = xmp.tile([D, K], dt.bfloat16, tag="xm", name=f"xm_{i}")
                xms[i] = xm
                nc.vector._custom_dve(
                    TENSOR_TENSOR_REDUCE,
                    out=xm[:, :],
                    in0=xt[:, :],
                    in1=mbSs[i][:, :],
                    s0=0.0,
                    s1=1.0,
                    accum_out=mean[:, i : i + 1],
                )
                if i == 0:
                    setup2()
            if j >= 0:
                for c in range(NCH):
                    nc.scalar.activation(
                        et[0:1, c * CH : (c + 1) * CH],
                        sts[c][:, :],
                        Act.Exp,
                        scale=SD,
                        accum_out=zp[0:1, NCH * j + c : NCH * j + c + 1],
                    )
                    eb = pp.tile(
                        [D, CH], dt.float32, tag="w1", name=f"eb_{j}_{c}"
                    )
                    for h in range(CH // 512):
                        lo = c * CH + h * 512
                        nc.tensor.matmul(
                            eb[:, h * 512 : (h + 1) * 512],
                            ones32[:, :],
                            et[:, lo : lo + 512],
                            start=True,
                            stop=True,
                        )
                    jt = junk.tile(
                        [D, CH], dt.bfloat16, tag="junk", name=f"j_{j}_{c}"
                    )
                    nc.vector._custom_dve(
                        TENSOR_TENSOR_REDUCE,
                        out=jt[:, :],
                        in0=xmj[:, c * CH : (c + 1) * CH],
                        in1=eb[:, :],
                        s0=0.0,
                        s1=1.0,
                        accum_out=praw4[:, NCH * j + c : NCH * j + c + 1],
                    )
            if i < NI:
                # v_i = (CQK^T . u_i) * (1/c_i) + w0
                nc.scalar.copy(meanb[:, i : i + 1], mean[:, i : i + 1])
                vps = pp.tile([D, 1], dt.float32, tag="w2", name=f"vps_{i}")
                nc.tensor.matmul(
                    vps[:, :],
                    cqk[:, :],
                    meanb[:, i : i + 1],
                    start=True,
                    stop=True,
                )
                nc.scalar.activation(
                    vt[:, i : i + 1],
                    vps[:, :],
                    Act.Identity,
                    bias=w0[:, 0:1],
                    scale=cinvb[:, i : i + 1],
                )

        mask_bcast(0)
        mask_bcast(1)
        prefetch(0)
        prefetch(1)
        for t in ets:
            nc.vector.memset(t[:, :], 0.0)
        for i in range(NI + 1):
            if i + 2 < NI:
                prefetch(i + 2)
            if i + 2 < NI:
                mask_bcast(i + 2)
            slot(i)

        # ---- finalize: out = praw / Z with Z = Zfake - (K - c) ----
        p4 = praw4[:, :].rearrange("p (n c) -> p n c", c=NCH)
        nc.vector.tensor_reduce(
            praw[:, :], p4, axis=mybir.AxisListType.X, op=Alu.add
        )
        zp3 = zp[:, :].rearrange("a (n c) -> a n c", c=NCH)
        nc.vector.tensor_reduce(
            zrow[:, :], zp3, axis=mybir.AxisListType.X, op=Alu.add
        )
        # zrow = (zfake + c) - K
        nc.vector.tensor_tensor(zrow[:, :], zrow[:, :], crow, op=Alu.add)
        nc.vector.tensor_scalar(
            zrow[:, :], zrow[:, :], -float(K), None, op0=Alu.add
        )
        nc.vector.reciprocal(zinv[:, :], zrow[:, :])
        nc.vector.tensor_copy(frow32[0:1, :], zinv[:, :])
        fb = pp.tile([D, NI], dt.float32, tag="w2")
        nc.tensor.matmul(
            fb[:, :], ones32f[:, :], frow32[:, :], start=True, stop=True
        )
        nc.vector.tensor_tensor(outt[:, :], praw[:, :], fb[:, :], op=Alu.mult)
        nc.sync.dma_start(out_d[:, :], outt[:, :])

    nc.compile()
    return nc


def _get_nc():
    if "nc" not in _CACHE:
        _CACHE["nc"] = _build()
    return _CACHE["nc"]


def kernel(x, mask, Wq, bq, Wk, bk):
    import ml_dtypes
    from concourse.bass_utils import run_bass_kernel_spmd

    nc = _get_nc()
    B, N, d, H, W = x.shape
    xr = np.ascontiguousarray(x.reshape(B * N, d, H * W).astype(np.float32))
    mr = mask.reshape(B * N, H * W)
    mrow = np.ascontiguousarray(mr.astype(ml_dtypes.bfloat16))
    cnt = np.asarray(mr).astype(np.float32).sum(axis=1)
    bq2 = np.ascontiguousarray(bq.reshape(d, 1).astype(np.float32))
    wqc = np.ascontiguousarray(Wq.astype(np.float32))
    wkc = np.ascontiguousarray(Wk.astype(np.float32))
    in_maps = []
    for c in range(NCORES):
        s = slice(c * NI, (c + 1) * NI)
        in_maps.append(
            {
                "x": np.ascontiguousarray(xr[s]),
                "mrow": np.ascontiguousarray(mrow[s]),
                "crow": np.ascontiguousarray(cnt[s].reshape(1, NI)),
                "cinv": np.ascontiguousarray((1.0 / cnt[s]).reshape(1, NI)),
                "Wq": wqc,
                "Wk": wkc,
                "bq": bq2,
            }
        )
    res = run_bass_kernel_spmd(nc, in_maps, core_ids=list(range(NCORES)))
    parts = [np.asarray(res.results[c]["out"]).T for c in range(NCORES)]
    return np.concatenate(parts, axis=0).reshape(B, N, d).astype(np.float32)
